# revision 24
# baseline (speedup 1.0000x reference)
"""Multi-head causal attention (B=4, S=2048, D=1024, H=16, E=64) on 8 TRN2 cores.

Sharding: core c handles batch b = c//2 and head-group g = c%2 (8 heads each).
Each core computes its batch's attention for its 8 heads plus the partial
output projection over its 512 feature columns; the host sums the two
head-group partials per batch and adds the bias.

Per-core dataflow (all matmuls fp32r = full PE rate at N=512), interleaved by
512-row s-block j so QKV(j+1) PE work overlaps attention(j) ACT work:
  x[s-block] --PE transpose--> xT slice --matmul--> QT_j/KT [e,s], V [s,e]
  scoresT[t,sq] = KT-slice.T @ QT-slice   (K=64, two heads row-packed)
  expS = exp(scores/8) on ACT, causal via chunk skipping + wedge mask
  attnV: lhsT = [V_h | ones] (M=65) -> psum rows 0:64 = out.T, row 64 = sums
  normalize: spread sums -> reciprocal+NR -> K=1 broadcast matmul -> multiply
  proj(j): y[s,i] += outT_j[j',s].T @ WpT[j',i] over 4 pair-chunks
"""

import numpy as np

B, S, D = 4, 2048, 1024
H, E = 16, 64  # global heads, head dim
HL = 8  # heads per core
P = 128
NPAIR = 4  # head pairs per core
DC = 8  # d chunks of 128
NSB = 4  # s-blocks of 512
NSC = 16  # s-chunks of 128
SCALE = 1.0 / np.sqrt(E)

_CACHE = {}


def _build_nc():
    import concourse.bass as bass  # noqa: F401
    import concourse.mybir as mybir
    import concourse.tile as tile
    from concourse import bacc

    F32 = mybir.dt.float32
    F32R = mybir.dt.float32r
    AF = mybir.ActivationFunctionType

    nc = bacc.Bacc(None, target_bir_lowering=False)

    x_d = nc.declare_dram_parameter("x", [D, S], F32R, isOutput=False)
    wq_d = nc.declare_dram_parameter("wq", [P, DC, 512], F32R, isOutput=False)
    wk_d = nc.declare_dram_parameter("wk", [P, DC, 512], F32R, isOutput=False)
    wv_d = nc.declare_dram_parameter("wv", [P, DC, 512], F32R, isOutput=False)
    wpt_d = nc.declare_dram_parameter("wpt", [P, NPAIR, D], F32R, isOutput=False)
    tri_d = nc.declare_dram_parameter("trimask", [P, P], F32R, isOutput=False)
    y_d = nc.declare_dram_parameter("y", [S, D], F32, isOutput=True)

    with tile.TileContext(nc) as tc:
        with (
            tc.tile_pool(name="const", bufs=1) as pconst,
            tc.tile_pool(name="ktp", bufs=1) as pkt,
            tc.tile_pool(name="vp", bufs=1) as pv_pool,
            tc.tile_pool(name="wp", bufs=1) as pw,
            tc.tile_pool(name="qtp", bufs=2) as pqt,
            tc.tile_pool(name="xtp", bufs=2) as pxt,
            tc.tile_pool(name="es", bufs=2) as pes,
            tc.tile_pool(name="rst", bufs=1) as prst,
            tc.tile_pool(name="otp", bufs=1) as pot,
            tc.tile_pool(name="wptp", bufs=1) as pwpt,
            tc.tile_pool(name="yout", bufs=1) as py,
            tc.tile_pool(name="psum", bufs=1, space="PSUM") as pps,
        ):
            tri_t = pconst.tile([P, P], F32R)
            nc.sync.dma_start(tri_t[:], tri_d[:])

            kt_t = pkt.tile([P, NPAIR, S], F32R)  # [e-in-pair, pair, t]
            v_t = pv_pool.tile([P, NSC, HL, 65], F32R)  # [s%128, s//128, h, e|1]
            # ones column of V: tri row-broadcast trick (tri*0 + 1)
            nc.vector.tensor_scalar(
                v_t[:, :, :, 64:65].rearrange("p a b c -> p (a b c)"),
                tri_t[:, 0:128], 0.0, 1.0,
                mybir.AluOpType.mult, mybir.AluOpType.add,
            )
            wq_t = pw.tile([P, DC, 512], F32R)
            wk_t = pw.tile([P, DC, 512], F32R)
            wv_t = pw.tile([P, DC, 512], F32R)
            nc.sync.dma_start(wq_t[:], wq_d[:])
            nc.sync.dma_start(wk_t[:], wk_d[:])
            nc.sync.dma_start(wv_t[:], wv_d[:])
            wpt_t = pwpt.tile([P, NPAIR, D], F32R)
            nc.sync.dma_start(wpt_t[:], wpt_d[:])

            xt_sl = {}
            qt_sl = {}

            def emit_xt_load(jj):
                nc.sync.dma_start(
                    xt_sl[jj % 2][:],
                    x_d[:, jj * 512 : (jj + 1) * 512].rearrange(
                        "(dc p) s -> p dc s", p=P
                    ),
                )

            def qk_step(jj, pr, w_t, dst_sel):
                def go():
                    xt_t = xt_sl[jj % 2]
                    pq = pps.tile([P, 512], F32, tag="mm", name="pq")
                    for dc in range(DC):
                        nc.tensor.matmul(
                            pq[:],
                            w_t[:, dc, pr * P : (pr + 1) * P],
                            xt_t[:, dc, :],
                            start=(dc == 0),
                            stop=(dc == DC - 1),
                        )
                    if dst_sel == "q":
                        nc.vector.tensor_copy(qt_sl[jj % 2][:, pr, :], pq[:])
                    else:
                        nc.vector.tensor_copy(
                            kt_t[:, pr, jj * 512 : (jj + 1) * 512], pq[:]
                        )
                return go

            def v_step(jj, sc):
                def go():
                    xt_t = xt_sl[jj % 2]
                    pvv = pps.tile([P, 512], F32, tag="mm", name="pvv")
                    for dc in range(DC):
                        nc.tensor.matmul(
                            pvv[:],
                            xt_t[:, dc, sc * P : (sc + 1) * P],
                            wv_t[:, dc, :],
                            start=(dc == 0),
                            stop=(dc == DC - 1),
                        )
                    nc.vector.tensor_copy(
                        v_t[:, jj * 4 + sc, :, 0:64],
                        pvv[:].rearrange("p (h e) -> p h e", e=64),
                    )
                return go

            def qkv_steps(jj):
                xt_sl[jj % 2] = pxt.tile(
                    [P, DC, 512], F32R, tag="xt", name="xtn"
                )
                emit_xt_load(jj)
                qt_sl[jj % 2] = pqt.tile(
                    [P, NPAIR, 512], F32R, tag="qt", name="qtn"
                )
                steps = []
                for pr in range(NPAIR):
                    steps.append(qk_step(jj, pr, wq_t, "q"))
                    steps.append(qk_step(jj, pr, wk_t, "k"))
                for sc in range(4):
                    steps.append(v_step(jj, sc))
                return steps

            def proj_steps(j, ot_t):
                def mk(sc):
                    def go():
                        s0 = j * 512 + sc * P
                        y_t = py.tile([P, D], F32, tag="y", name="yt")
                        pp0 = pps.tile([P, 512], F32, tag="mm", name="pp0")
                        pp1 = pps.tile([P, 512], F32, tag="mm", name="pp1")
                        for pr in range(NPAIR):
                            for ib, pp in ((0, pp0), (1, pp1)):
                                nc.tensor.matmul(
                                    pp[:],
                                    ot_t[:, pr, sc * P : (sc + 1) * P],
                                    wpt_t[:, pr, ib * 512 : (ib + 1) * 512],
                                    start=(pr == 0),
                                    stop=(pr == NPAIR - 1),
                                )
                        nc.vector.tensor_copy(y_t[:, 0:512], pp0[:])
                        nc.vector.tensor_copy(y_t[:, 512:1024], pp1[:])
                        nc.sync.dma_start(y_d[s0 : s0 + P, :], y_t[:])
                    return go
                return [mk(sc) for sc in range(4)]

            # block 0 QKV up front (dense)
            for step in qkv_steps(0):
                step()

            fill = []
            for j in range(NSB):
                qt_t = qt_sl[j % 2]
                if j < NSB - 1:
                    fill.extend(qkv_steps(j + 1))
                total_chunks = (4 * j + 4) * NPAIR
                done_chunks = 0
                ot_t = pot.tile([P, NPAIR, 512], F32R, tag="ot", name="otn")
                for pr in range(NPAIR):
                    pv = pps.tile([P, 2, 512], F32, tag="av", name="pvn")
                    nchunk = 4 * j + 4
                    for i in range(nchunk):
                        t_sl = slice(i * P, (i + 1) * P)
                        delta = i * P - j * 512
                        d0 = max(delta, 0)
                        psc = pps.tile([P, 2, 512], F32, tag="sc", name="pscn")
                        nc.tensor.matmul(
                            psc[:, 0, d0:512],
                            kt_t[0:64, pr, t_sl],
                            qt_t[0:64, pr, d0:512],
                            start=True, stop=True, tile_position=(0, 0),
                        )
                        nc.tensor.matmul(
                            psc[:, 1, d0:512],
                            kt_t[64:128, pr, t_sl],
                            qt_t[64:128, pr, d0:512],
                            start=True, stop=True, tile_position=(64, 0),
                        )
                        es = pes.tile([P, 2, 512], F32R, tag="es", name="esn")
                        nc.scalar.activation(
                            es[:, :, d0:512],
                            psc[:, :, d0:512],
                            AF.Exp,
                            scale=float(SCALE),
                        )
                        if delta >= 0:
                            for hl in range(2):
                                nc.vector.tensor_mul(
                                    es[:, hl, delta : delta + P],
                                    es[:, hl, delta : delta + P],
                                    tri_t[:],
                                )
                        for hl in range(2):
                            nc.tensor.matmul(
                                pv[0:65, hl, d0:512],
                                v_t[:, i, 2 * pr + hl, 0:65],
                                es[:, hl, d0:512],
                                start=(i == 0),
                                stop=(i == nchunk - 1),
                            )
                        # pump filler so PE never drains
                        done_chunks += 1
                        rem = total_chunks - done_chunks
                        import math as _m
                        want = (
                            len(fill) if rem == 0
                            else _m.ceil(len(fill) / (rem + 1))
                        )
                        for _ in range(min(want, len(fill))):
                            fill.pop(0)()
                    # drain pv fast, then normalize off the critical path
                    st0 = prst.tile([65, 512], F32R, tag="st0")
                    nc.vector.tensor_copy(st0[:], pv[0:65, 0, :])
                    st1 = prst.tile([65, 512], F32R, tag="st1")
                    nc.vector.tensor_copy(st1[:], pv[0:65, 1, :])
                    sp = prst.tile([P, 32], F32, tag="sp")
                    d_sl = sp[:, 0:8]
                    r0_sl = sp[:, 8:16]
                    t_sl2 = sp[:, 16:24]
                    r1_sl = sp[:, 24:32]
                    nc.sync.dma_start(d_sl[0:64, :], st0[64:65, :].bitcast(F32))
                    nc.sync.dma_start(d_sl[64:128, :], st1[64:65, :].bitcast(F32))
                    nc.vector.reciprocal(r0_sl, d_sl)
                    nc.vector.tensor_mul(t_sl2, r0_sl, d_sl)
                    nc.vector.tensor_scalar(
                        t_sl2, t_sl2, -1.0, 2.0,
                        mybir.AluOpType.mult, mybir.AluOpType.add,
                    )
                    nc.vector.tensor_mul(r1_sl, r0_sl, t_sl2)
                    rr = prst.tile([1, 1024], F32R, tag="rr")
                    nc.sync.dma_start(rr[0:1, :], r1_sl.bitcast(F32R))
                    pbc0 = pps.tile([P, 512], F32, tag="mm", name="pbc0")
                    nc.tensor.matmul(
                        pbc0[0:64, :], tri_t[0:1, 0:64], rr[0:1, 0:512],
                        start=True, stop=True,
                    )
                    pbc1 = pps.tile([P, 512], F32, tag="mm", name="pbc1")
                    nc.tensor.matmul(
                        pbc1[0:64, :], tri_t[0:1, 0:64], rr[0:1, 512:1024],
                        start=True, stop=True,
                    )
                    nc.vector.tensor_mul(
                        ot_t[0:64, pr, :], st0[0:64, :], pbc0[0:64, :]
                    )
                    nc.vector.tensor_mul(
                        st1[0:64, :], st1[0:64, :], pbc1[0:64, :]
                    )
                    nc.sync.dma_start(ot_t[64:128, pr, :], st1[0:64, :])
                # projection queued as filler for the next block
                if j < NSB - 1:
                    fill.extend(proj_steps(j, ot_t))
                else:
                    for step in proj_steps(j, ot_t):
                        step()
            for step in fill:
                step()

    nc.compile()
    return nc


def _host_inputs(Wq, Wk, Wv, Wp):
    """Per-head-group device weight layouts."""

    def wdev(W, g):
        # W [16, 1024, 64] -> local [8, D, E] -> [D, 512] -> [P, DC, 512]
        Ws = W[g * HL : (g + 1) * HL]  # [8, D, E]
        A = Ws.transpose(1, 0, 2).reshape(D, HL * E)  # [d, h*64+e]
        return np.ascontiguousarray(
            A.reshape(DC, P, HL * E).transpose(1, 0, 2)
        ).astype(np.float32)

    def wptdev(Wp, g):
        # Wp [D, D]; j slice -> [512, D] -> [P, NPAIR, D]
        A = Wp[:, g * 512 : (g + 1) * 512].T  # [j, i]
        return np.ascontiguousarray(
            A.reshape(NPAIR, P, D).transpose(1, 0, 2)
        ).astype(np.float32)

    out = {}
    for g in range(2):
        out[g] = {
            "wq": wdev(Wq, g),
            "wk": wdev(Wk, g),
            "wv": wdev(Wv, g),
            "wpt": wptdev(Wp, g),
        }
    return out


def _consts():
    iot = np.arange(P)
    trimask = (iot[:, None] <= iot[None, :]).astype(np.float32)
    ones128 = np.ones((P, P), np.float32)
    return {
        "trimask": trimask,
        "ones128": ones128,
    }


def kernel(x, Wq, Wk, Wv, Wp, bp):
    from concourse.bass_utils import run_bass_kernel_spmd

    x = np.asarray(x, dtype=np.float32)
    Wq = np.asarray(Wq, dtype=np.float32)
    Wk = np.asarray(Wk, dtype=np.float32)
    Wv = np.asarray(Wv, dtype=np.float32)
    Wp = np.asarray(Wp, dtype=np.float32)
    bp = np.asarray(bp, dtype=np.float32)

    if "nc" not in _CACHE:
        _CACHE["nc"] = _build_nc()
    nc = _CACHE["nc"]

    wmaps = _host_inputs(Wq, Wk, Wv, Wp)
    consts = _consts()
    in_maps = []
    for c in range(8):
        b, g = c // 2, c % 2
        m = {"x": np.ascontiguousarray(x[b].T)}
        m.update(wmaps[g])
        m.update(consts)
        in_maps.append(m)

    res = run_bass_kernel_spmd(nc, in_maps, list(range(8)))
    out = np.empty((B, S, D), np.float32)
    for b in range(B):
        out[b] = res.results[2 * b]["y"] + res.results[2 * b + 1]["y"] + bp
    return out


# revision 25
# speedup vs baseline: 1.1008x; 1.1008x over previous
"""Multi-head causal attention (B=4, S=2048, D=1024, H=16, E=64) on 8 TRN2 cores.

Sharding: core c handles batch b = c//2 and head-group g = c%2 (8 heads each).
Each core computes its batch's attention for its 8 heads plus the partial
output projection over its 512 feature columns; the host sums the two
head-group partials per batch and adds the bias.

Per-core dataflow (all matmuls fp32r = full PE rate at N=512), interleaved by
512-row s-block j so QKV(j+1) PE work overlaps attention(j) ACT work:
  x[s-block] --PE transpose--> xT slice --matmul--> QT_j/KT [e,s], V [s,e]
  scoresT[t,sq] = KT-slice.T @ QT-slice   (K=64, two heads row-packed)
  expS = exp(scores/8) on ACT, causal via chunk skipping + wedge mask
  attnV: lhsT = [V_h | ones] (M=65) -> psum rows 0:64 = out.T, row 64 = sums
  normalize: spread sums -> reciprocal+NR -> K=1 broadcast matmul -> multiply
  proj(j): y[s,i] += outT_j[j',s].T @ WpT[j',i] over 4 pair-chunks
"""

import numpy as np

B, S, D = 4, 2048, 1024
H, E = 16, 64  # global heads, head dim
HL = 8  # heads per core
P = 128
NPAIR = 4  # head pairs per core
DC = 8  # d chunks of 128
NSB = 4  # s-blocks of 512
NSC = 16  # s-chunks of 128
SCALE = 1.0 / np.sqrt(E)

_CACHE = {}


def _build_nc():
    import concourse.bass as bass  # noqa: F401
    import concourse.mybir as mybir
    import concourse.tile as tile
    from concourse import bacc

    F32 = mybir.dt.float32
    F32R = mybir.dt.float32r
    AF = mybir.ActivationFunctionType

    nc = bacc.Bacc(None, target_bir_lowering=False)

    x_d = nc.declare_dram_parameter("x", [D, S], F32R, isOutput=False)
    wq_d = nc.declare_dram_parameter("wq", [P, DC, 512], F32R, isOutput=False)
    wk_d = nc.declare_dram_parameter("wk", [P, DC, 512], F32R, isOutput=False)
    wv_d = nc.declare_dram_parameter("wv", [P, DC, 512], F32R, isOutput=False)
    wpt_d = nc.declare_dram_parameter("wpt", [P, NPAIR, D], F32R, isOutput=False)
    tri_d = nc.declare_dram_parameter("trimask", [P, P], F32R, isOutput=False)
    y_d = nc.declare_dram_parameter("y", [S, D], F32, isOutput=True)

    with tile.TileContext(nc) as tc:
        with (
            tc.tile_pool(name="const", bufs=1) as pconst,
            tc.tile_pool(name="ktp", bufs=1) as pkt,
            tc.tile_pool(name="vp", bufs=1) as pv_pool,
            tc.tile_pool(name="wp", bufs=1) as pw,
            tc.tile_pool(name="qtp", bufs=2) as pqt,
            tc.tile_pool(name="xtp", bufs=1) as pxt,
            tc.tile_pool(name="es", bufs=4) as pes,
            tc.tile_pool(name="rst", bufs=1) as prst,
            tc.tile_pool(name="otp", bufs=1) as pot,
            tc.tile_pool(name="wptp", bufs=1) as pwpt,
            tc.tile_pool(name="yout", bufs=1) as py,
            tc.tile_pool(name="psum", bufs=3, space="PSUM") as pps,
            tc.tile_pool(name="psav", bufs=1, space="PSUM") as ppsav,
        ):
            tri_t = pconst.tile([P, P], F32R)
            nc.sync.dma_start(tri_t[:], tri_d[:])

            kt_t = pkt.tile([P, NPAIR, S], F32R)  # [e-in-pair, pair, t]
            v_t = pv_pool.tile([P, NSC, HL, 65], F32R)  # [s%128, s//128, h, e|1]
            # ones column of V: tri row-broadcast trick (tri*0 + 1)
            nc.vector.tensor_scalar(
                v_t[:, :, :, 64:65].rearrange("p a b c -> p (a b c)"),
                tri_t[:, 0:128], 0.0, 1.0,
                mybir.AluOpType.mult, mybir.AluOpType.add,
            )
            wq_t = pw.tile([P, DC, 512], F32R)
            wk_t = pw.tile([P, DC, 512], F32R)
            wv_t = pw.tile([P, DC, 512], F32R)
            nc.sync.dma_start(wq_t[:], wq_d[:])
            nc.sync.dma_start(wk_t[:], wk_d[:])
            nc.sync.dma_start(wv_t[:], wv_d[:])
            wpt_t = pwpt.tile([P, NPAIR, D], F32R)
            nc.sync.dma_start(wpt_t[:], wpt_d[:])

            xt_sl = {}
            qt_sl = {}

            def emit_xt_load(jj):
                nc.sync.dma_start(
                    xt_sl[jj % 2][:],
                    x_d[:, jj * 512 : (jj + 1) * 512].rearrange(
                        "(dc p) s -> p dc s", p=P
                    ),
                )

            def qk_step(jj, pr, w_t, dst_sel):
                def go():
                    xt_t = xt_sl[jj % 2]
                    pqs = pps.tile([P, 2, 512], F32, tag="big", name="pq")
                    pq = pqs[:, 0, :]
                    for dc in range(DC):
                        nc.tensor.matmul(
                            pq[:],
                            w_t[:, dc, pr * P : (pr + 1) * P],
                            xt_t[:, dc, :],
                            start=(dc == 0),
                            stop=(dc == DC - 1),
                        )
                    if dst_sel == "q":
                        nc.vector.tensor_copy(qt_sl[jj % 2][:, pr, :], pq[:])
                    else:
                        nc.vector.tensor_copy(
                            kt_t[:, pr, jj * 512 : (jj + 1) * 512], pq[:]
                        )
                return go

            def v_step(jj, sc):
                def go():
                    xt_t = xt_sl[jj % 2]
                    pvs = pps.tile([P, 2, 512], F32, tag="big", name="pvv")
                    pvv = pvs[:, 0, :]
                    for dc in range(DC):
                        nc.tensor.matmul(
                            pvv[:],
                            xt_t[:, dc, sc * P : (sc + 1) * P],
                            wv_t[:, dc, :],
                            start=(dc == 0),
                            stop=(dc == DC - 1),
                        )
                    nc.vector.tensor_copy(
                        v_t[:, jj * 4 + sc, :, 0:64],
                        pvv[:].rearrange("p (h e) -> p h e", e=64),
                    )
                return go

            def qkv_steps(jj):
                xt_sl[jj % 2] = pxt.tile(
                    [P, DC, 512], F32R, tag="xt", name="xtn"
                )
                emit_xt_load(jj)
                qt_sl[jj % 2] = pqt.tile(
                    [P, NPAIR, 512], F32R, tag="qt", name="qtn"
                )
                steps = []
                for pr in range(NPAIR):
                    steps.append(qk_step(jj, pr, wq_t, "q"))
                    steps.append(qk_step(jj, pr, wk_t, "k"))
                for sc in range(4):
                    steps.append(v_step(jj, sc))
                return steps

            def proj_steps(j, ot_t):
                def mk(sc):
                    def go():
                        s0 = j * 512 + sc * P
                        y_t = py.tile([P, D], F32, tag="y", name="yt")
                        pps_ = pps.tile([P, 2, 512], F32, tag="big", name="ppn")
                        pp0 = pps_[:, 0, :]
                        pp1 = pps_[:, 1, :]
                        for pr in range(NPAIR):
                            for ib, pp in ((0, pp0), (1, pp1)):
                                nc.tensor.matmul(
                                    pp[:],
                                    ot_t[:, pr, sc * P : (sc + 1) * P],
                                    wpt_t[:, pr, ib * 512 : (ib + 1) * 512],
                                    start=(pr == 0),
                                    stop=(pr == NPAIR - 1),
                                )
                        nc.vector.tensor_copy(y_t[:, 0:512], pp0[:])
                        nc.vector.tensor_copy(y_t[:, 512:1024], pp1[:])
                        nc.sync.dma_start(y_d[s0 : s0 + P, :], y_t[:])
                    return go
                return [mk(sc) for sc in range(4)]

            # block 0 QKV up front (dense)
            for step in qkv_steps(0):
                step()

            fill = []
            for j in range(NSB):
                qt_t = qt_sl[j % 2]
                if j < NSB - 1:
                    fill.extend(qkv_steps(j + 1))
                total_chunks = (4 * j + 4) * NPAIR
                done_chunks = 0
                ot_t = pot.tile([P, NPAIR, 512], F32R, tag="ot", name="otn")
                for pr in range(NPAIR):
                    pv = ppsav.tile([P, 2, 512], F32, tag="av", name="pvn")
                    nchunk = 4 * j + 4
                    for i in range(nchunk):
                        t_sl = slice(i * P, (i + 1) * P)
                        delta = i * P - j * 512
                        d0 = max(delta, 0)
                        psc = pps.tile([P, 2, 512], F32, tag="big", name="pscn")
                        nc.tensor.matmul(
                            psc[:, 0, d0:512],
                            kt_t[0:64, pr, t_sl],
                            qt_t[0:64, pr, d0:512],
                            start=True, stop=True, tile_position=(0, 0),
                        )
                        nc.tensor.matmul(
                            psc[:, 1, d0:512],
                            kt_t[64:128, pr, t_sl],
                            qt_t[64:128, pr, d0:512],
                            start=True, stop=True, tile_position=(64, 0),
                        )
                        es = pes.tile([P, 2, 512], F32R, tag="es", name="esn")
                        nc.scalar.activation(
                            es[:, :, d0:512],
                            psc[:, :, d0:512],
                            AF.Exp,
                            scale=float(SCALE),
                        )
                        if delta >= 0:
                            for hl in range(2):
                                nc.vector.tensor_mul(
                                    es[:, hl, delta : delta + P],
                                    es[:, hl, delta : delta + P],
                                    tri_t[:],
                                )
                        for hl in range(2):
                            nc.tensor.matmul(
                                pv[0:65, hl, d0:512],
                                v_t[:, i, 2 * pr + hl, 0:65],
                                es[:, hl, d0:512],
                                start=(i == 0),
                                stop=(i == nchunk - 1),
                            )
                        # pump filler so PE never drains
                        done_chunks += 1
                        rem = total_chunks - done_chunks
                        import math as _m
                        want = (
                            len(fill) if rem == 0
                            else _m.ceil(len(fill) / (rem + 1))
                        )
                        for _ in range(min(want, len(fill))):
                            fill.pop(0)()
                    # drain pv fast, then normalize off the critical path
                    st0 = prst.tile([65, 512], F32R, tag="st0")
                    nc.vector.tensor_copy(st0[:], pv[0:65, 0, :])
                    st1 = prst.tile([65, 512], F32R, tag="st1")
                    nc.vector.tensor_copy(st1[:], pv[0:65, 1, :])
                    sp = prst.tile([P, 32], F32, tag="sp")
                    d_sl = sp[:, 0:8]
                    r0_sl = sp[:, 8:16]
                    t_sl2 = sp[:, 16:24]
                    r1_sl = sp[:, 24:32]
                    nc.sync.dma_start(d_sl[0:64, :], st0[64:65, :].bitcast(F32))
                    nc.sync.dma_start(d_sl[64:128, :], st1[64:65, :].bitcast(F32))
                    nc.vector.reciprocal(r0_sl, d_sl)
                    nc.vector.tensor_mul(t_sl2, r0_sl, d_sl)
                    nc.vector.tensor_scalar(
                        t_sl2, t_sl2, -1.0, 2.0,
                        mybir.AluOpType.mult, mybir.AluOpType.add,
                    )
                    nc.vector.tensor_mul(r1_sl, r0_sl, t_sl2)
                    rr = prst.tile([1, 1024], F32R, tag="rr")
                    nc.sync.dma_start(rr[0:1, :], r1_sl.bitcast(F32R))
                    pbcs = pps.tile([P, 2, 512], F32, tag="big", name="pbcn")
                    pbc0 = pbcs[:, 0, :]
                    pbc1 = pbcs[:, 1, :]
                    nc.tensor.matmul(
                        pbc0[0:64, :], tri_t[0:1, 0:64], rr[0:1, 0:512],
                        start=True, stop=True,
                    )
                    nc.tensor.matmul(
                        pbc1[0:64, :], tri_t[0:1, 0:64], rr[0:1, 512:1024],
                        start=True, stop=True,
                    )
                    nc.vector.tensor_mul(
                        ot_t[0:64, pr, :], st0[0:64, :], pbc0[0:64, :]
                    )
                    nc.vector.tensor_mul(
                        st1[0:64, :], st1[0:64, :], pbc1[0:64, :]
                    )
                    nc.sync.dma_start(ot_t[64:128, pr, :], st1[0:64, :])
                # projection queued as filler for the next block
                if j < NSB - 1:
                    fill.extend(proj_steps(j, ot_t))
                else:
                    for step in proj_steps(j, ot_t):
                        step()
            for step in fill:
                step()

    nc.compile()
    return nc


def _host_inputs(Wq, Wk, Wv, Wp):
    """Per-head-group device weight layouts."""

    def wdev(W, g):
        # W [16, 1024, 64] -> local [8, D, E] -> [D, 512] -> [P, DC, 512]
        Ws = W[g * HL : (g + 1) * HL]  # [8, D, E]
        A = Ws.transpose(1, 0, 2).reshape(D, HL * E)  # [d, h*64+e]
        return np.ascontiguousarray(
            A.reshape(DC, P, HL * E).transpose(1, 0, 2)
        ).astype(np.float32)

    def wptdev(Wp, g):
        # Wp [D, D]; j slice -> [512, D] -> [P, NPAIR, D]
        A = Wp[:, g * 512 : (g + 1) * 512].T  # [j, i]
        return np.ascontiguousarray(
            A.reshape(NPAIR, P, D).transpose(1, 0, 2)
        ).astype(np.float32)

    out = {}
    for g in range(2):
        out[g] = {
            "wq": wdev(Wq, g),
            "wk": wdev(Wk, g),
            "wv": wdev(Wv, g),
            "wpt": wptdev(Wp, g),
        }
    return out


def _consts():
    iot = np.arange(P)
    trimask = (iot[:, None] <= iot[None, :]).astype(np.float32)
    ones128 = np.ones((P, P), np.float32)
    return {
        "trimask": trimask,
        "ones128": ones128,
    }


def kernel(x, Wq, Wk, Wv, Wp, bp):
    from concourse.bass_utils import run_bass_kernel_spmd

    x = np.asarray(x, dtype=np.float32)
    Wq = np.asarray(Wq, dtype=np.float32)
    Wk = np.asarray(Wk, dtype=np.float32)
    Wv = np.asarray(Wv, dtype=np.float32)
    Wp = np.asarray(Wp, dtype=np.float32)
    bp = np.asarray(bp, dtype=np.float32)

    if "nc" not in _CACHE:
        _CACHE["nc"] = _build_nc()
    nc = _CACHE["nc"]

    wmaps = _host_inputs(Wq, Wk, Wv, Wp)
    consts = _consts()
    in_maps = []
    for c in range(8):
        b, g = c // 2, c % 2
        m = {"x": np.ascontiguousarray(x[b].T)}
        m.update(wmaps[g])
        m.update(consts)
        in_maps.append(m)

    res = run_bass_kernel_spmd(nc, in_maps, list(range(8)))
    out = np.empty((B, S, D), np.float32)
    for b in range(B):
        out[b] = res.results[2 * b]["y"] + res.results[2 * b + 1]["y"] + bp
    return out
